# revision 1
# baseline (speedup 1.0000x reference)
"""Windowed cross-attention with relative position encodings, data-parallel
over batch across 8 NeuronCores.

Sharding (per spec hint): B=32 is split 4-per-core across the 8 cores;
the small q/kv/proj weights and the 169x1152 RPE table are replicated.
Windows are independent so attention needs no cross-device communication.

The RPE gather (static 169 -> [49,49] index table) is folded on the host
into dense per-(i,j,head) tables so each core runs pure einsum/softmax work.
"""

import functools

import numpy as np

import jax
import jax.numpy as jnp

WS = 7
NH = 12
DIM = 384
HD = DIM // NH
L = WS * WS
SCALE = HD ** (-0.5)
N_CORES = 8


def _relative_position_index() -> np.ndarray:
    coords = np.stack(np.meshgrid(np.arange(WS), np.arange(WS), indexing="ij"))
    flat = coords.reshape(2, -1)
    rel = flat[:, :, None] - flat[:, None, :]
    rel = rel.transpose(1, 2, 0).copy()
    rel[:, :, 0] += WS - 1
    rel[:, :, 1] += WS - 1
    rel[:, :, 0] *= 2 * WS - 1
    return rel.sum(-1)  # [L, L] int


_RPI = _relative_position_index()


def _partition(t, b, h, w):
    nh, nw = h // WS, w // WS
    t = t.reshape(b, nh, WS, nw, WS, NH, HD)
    t = t.transpose(0, 1, 3, 5, 2, 4, 6)
    return t.reshape(b * nh * nw, NH, L, HD)


def _unpartition(t, b, h, w):
    nh, nw = h // WS, w // WS
    t = t.reshape(b, nh, nw, NH, WS, WS, HD)
    t = t.transpose(0, 1, 4, 2, 5, 3, 6)
    return t.reshape(b, h, w, DIM)


def _core_fn(x, context, q_w, q_b, kv_w, kv_b, proj_w, proj_b,
             q_rpe, k_rpe, v_rpe):
    b, h, w, _ = x.shape
    q = x @ q_w + q_b
    kv = context @ kv_w + kv_b
    k, v = jnp.split(kv, 2, axis=-1)

    q = _partition(q, b, h, w) * SCALE
    k = _partition(k, b, h, w)
    v = _partition(v, b, h, w)

    qk = jnp.einsum("bhic,bhjc->bhij", q, k)
    qr = jnp.einsum("bhic,ijhc->bhij", q, k_rpe)
    kr = jnp.einsum("bhjc,ijhc->bhij", k, q_rpe)
    attn = jax.nn.softmax(qk + qr + kr, axis=-1)

    out = jnp.einsum("bhij,bhjc->bhic", attn, v) + jnp.einsum(
        "bhij,ijhc->bhic", attn, v_rpe
    )
    out = _unpartition(out, b, h, w)
    return out @ proj_w + proj_b


_PMAP = None


def _get_pmap():
    global _PMAP
    if _PMAP is None:
        _PMAP = jax.pmap(_core_fn, devices=jax.devices()[:N_CORES])
    return _PMAP


def _tile8(a):
    a = np.asarray(a)
    return np.broadcast_to(a, (N_CORES,) + a.shape)


def kernel(x, context, rpe_table, q_w, q_b, kv_w, kv_b, proj_w, proj_b):
    x = np.asarray(x)
    context = np.asarray(context)
    B, H, W, _ = x.shape
    per = B // N_CORES

    # host-side fold of the static gather: [169, 1152] -> three [L,L,NH,HD]
    rpe = np.asarray(rpe_table)[_RPI.reshape(-1)].reshape(L, L, NH, 3 * HD)
    q_rpe, k_rpe, v_rpe = np.split(rpe, 3, axis=-1)
    q_rpe = (q_rpe * SCALE).astype(np.float32)
    k_rpe = np.ascontiguousarray(k_rpe, dtype=np.float32)
    v_rpe = np.ascontiguousarray(v_rpe, dtype=np.float32)

    xs = x.reshape(N_CORES, per, H, W, DIM)
    cs = context.reshape(N_CORES, per, H, W, DIM)

    out = _get_pmap()(
        xs, cs,
        _tile8(q_w), _tile8(q_b),
        _tile8(kv_w), _tile8(kv_b),
        _tile8(proj_w), _tile8(proj_b),
        _tile8(q_rpe), _tile8(k_rpe), _tile8(v_rpe),
    )
    out = np.asarray(out).reshape(B, H, W, DIM)
    return out.astype(np.float32)



# revision 2
# speedup vs baseline: 1.0527x; 1.0527x over previous
"""Windowed cross-attention with contextual relative position encodings,
data-parallel over batch across 8 NeuronCores.

v2: neuron-compiler-friendly restructuring of the baseline pmap graph.
  - windows are formed BEFORE the projections (one 6D transpose of the
    f32 inputs instead of three on q/k/v),
  - the RPE einsums 'bhic,ijhc->bhij' are rewritten as clean batched
    matmuls over a (head, position) batch axis with HOST-prepared
    operand layouts (the dense [588, 32, 49] tables are inputs),
  - all matmuls run with bf16 operands and fp32 accumulation
    (preferred_element_type) -- fp32 matmuls are 4x slower on the PE
    array; softmax stays in fp32.  Tolerance is 2e-2 absmax-relative.
"""

import numpy as np

import jax
import jax.numpy as jnp

WS = 7
NH = 12
DIM = 384
HD = DIM // NH
L = WS * WS
SCALE = HD ** (-0.5)
N_CORES = 8
NW = 8  # windows per image side (56 / 7)


def _relative_position_index() -> np.ndarray:
    coords = np.stack(np.meshgrid(np.arange(WS), np.arange(WS), indexing="ij"))
    flat = coords.reshape(2, -1)
    rel = flat[:, :, None] - flat[:, None, :]
    rel = rel.transpose(1, 2, 0).copy()
    rel[:, :, 0] += WS - 1
    rel[:, :, 1] += WS - 1
    rel[:, :, 0] *= 2 * WS - 1
    return rel.sum(-1)  # [L, L] int


_RPI = _relative_position_index()

_BF = jnp.bfloat16
_F32 = jnp.float32


def _window(t, b):
    # [b,56,56,DIM] -> [b*8*8, L, DIM]
    t = t.reshape(b, NW, WS, NW, WS, DIM)
    t = t.transpose(0, 1, 3, 2, 4, 5)
    return t.reshape(b * NW * NW, L, DIM)


def _unwindow(t, b):
    # [b*8*8, L, DIM] -> [b,56,56,DIM]
    t = t.reshape(b, NW, NW, WS, WS, DIM)
    t = t.transpose(0, 1, 3, 2, 4, 5)
    return t.reshape(b, NW * WS, NW * WS, DIM)


def _bmm(a, b):
    # batched matmul, bf16 in / fp32 out
    return jax.lax.dot_general(
        a.astype(_BF), b.astype(_BF),
        (((a.ndim - 1,), (1,)), (tuple(range(a.ndim - 2)), (0,))),
        preferred_element_type=_F32,
    )


def _core_fn(x, context, q_w, q_b, k_w, k_b, v_w, v_b, proj_w, proj_b,
             k_rpe_b, q_rpe_b, v_rpe_b):
    b = x.shape[0]
    bw = b * NW * NW                       # windows on this core

    xw = _window(x, b).reshape(bw * L, DIM)
    cw = _window(context, b).reshape(bw * L, DIM)

    # projections (bf16 x bf16 -> fp32)
    mm = lambda a, w: jax.lax.dot_general(
        a.astype(_BF), w, (((1,), (0,)), ((), ())),
        preferred_element_type=_F32)
    q = mm(xw, q_w) * SCALE + q_b * SCALE      # [bw*L, DIM]
    k = mm(cw, k_w) + k_b
    v = (mm(cw, v_w) + v_b).astype(_BF)

    heads = lambda t: t.reshape(bw, L, NH, HD).transpose(0, 2, 1, 3)
    qh = heads(q)                               # [bw, NH, L, HD] fp32
    kh = heads(k)
    vh = heads(v)                               # bf16

    # qk[w,h,i,j]
    qk = _bmm(qh.reshape(bw * NH, L, HD),
              kh.reshape(bw * NH, L, HD).transpose(0, 2, 1))

    # qr[w,h,i,j] = sum_c q[w,h,i,c] * ktab[h,i,c,j]   (batch = h*i)
    q_hi = qh.transpose(1, 2, 0, 3).reshape(NH * L, bw, HD)
    qr = _bmm(q_hi, k_rpe_b)                    # [NH*L, bw, L]
    qr = qr.reshape(NH, L, bw, L).transpose(2, 0, 1, 3)

    # kr[w,h,i,j] = sum_c k[w,h,j,c] * qtab[h,j,c,i]   (batch = h*j)
    k_hj = kh.transpose(1, 2, 0, 3).reshape(NH * L, bw, HD)
    kr = _bmm(k_hj, q_rpe_b)                    # [NH*L(j), bw, L(i)]
    kr = kr.reshape(NH, L, bw, L).transpose(2, 0, 3, 1)

    logits = (qk.reshape(bw, NH, L, L) + qr + kr)
    attn = jax.nn.softmax(logits, axis=-1)      # fp32 [bw, NH, L, L]

    # out1 = attn @ v
    out1 = _bmm(attn.reshape(bw * NH, L, L), vh.reshape(bw * NH, L, HD))

    # out2[w,h,i,c] = sum_j attn[w,h,i,j] * vtab[h,i,j,c]  (batch = h*i)
    a_hi = attn.transpose(1, 2, 0, 3).reshape(NH * L, bw, L)
    out2 = _bmm(a_hi, v_rpe_b)                  # [NH*L, bw, HD]
    out2 = out2.reshape(NH, L, bw, HD).transpose(2, 0, 1, 3)

    out = out1.reshape(bw, NH, L, HD) + out2
    out = out.transpose(0, 2, 1, 3).reshape(bw * L, DIM)

    res = mm(out, proj_w) + proj_b
    return _unwindow(res.reshape(bw, L, DIM), b)


_PMAP = None


def _get_pmap():
    global _PMAP
    if _PMAP is None:
        _PMAP = jax.pmap(_core_fn, devices=jax.devices()[:N_CORES])
    return _PMAP


def _tile8(a):
    a = np.asarray(a)
    return np.broadcast_to(a, (N_CORES,) + a.shape)


def _prep_consts(rpe_table, q_w, q_b, kv_w, kv_b, proj_w, proj_b):
    """Host-side: fold the static RPE gather + lay out all constants."""
    import ml_dtypes

    rpe = np.asarray(rpe_table)[_RPI.reshape(-1)].reshape(L, L, NH, 3 * HD)
    q_rpe, k_rpe, v_rpe = np.split(rpe, 3, axis=-1)   # [i,j,h,c] each
    q_rpe = q_rpe * SCALE

    def as_bf16(a):
        return np.ascontiguousarray(a, np.float32).astype(ml_dtypes.bfloat16)

    # qr batch (h,i): ktab[h,i,c,j]
    k_rpe_b = as_bf16(k_rpe.transpose(2, 0, 3, 1).reshape(NH * L, HD, L))
    # kr batch (h,j): qtab[h,j,c,i]
    q_rpe_b = as_bf16(q_rpe.transpose(2, 1, 3, 0).reshape(NH * L, HD, L))
    # out2 batch (h,i): vtab[h,i,j,c]
    v_rpe_b = as_bf16(v_rpe.transpose(2, 0, 1, 3).reshape(NH * L, L, HD))

    kv_w = np.asarray(kv_w)
    kv_b = np.asarray(kv_b)
    consts = dict(
        q_w=as_bf16(q_w), q_b=np.asarray(q_b, np.float32),
        k_w=as_bf16(kv_w[:, :DIM]), k_b=kv_b[:DIM].astype(np.float32),
        v_w=as_bf16(kv_w[:, DIM:]), v_b=kv_b[DIM:].astype(np.float32),
        proj_w=as_bf16(proj_w), proj_b=np.asarray(proj_b, np.float32),
        k_rpe_b=k_rpe_b, q_rpe_b=q_rpe_b, v_rpe_b=v_rpe_b,
    )
    return consts


def kernel(x, context, rpe_table, q_w, q_b, kv_w, kv_b, proj_w, proj_b):
    x = np.asarray(x)
    context = np.asarray(context)
    B, H, W, _ = x.shape
    per = B // N_CORES

    consts = _prep_consts(rpe_table, q_w, q_b, kv_w, kv_b, proj_w, proj_b)

    xs = x.reshape(N_CORES, per, H, W, DIM)
    cs = context.reshape(N_CORES, per, H, W, DIM)

    out = _get_pmap()(
        xs, cs,
        *[_tile8(consts[n]) for n in
          ("q_w", "q_b", "k_w", "k_b", "v_w", "v_b", "proj_w", "proj_b",
           "k_rpe_b", "q_rpe_b", "v_rpe_b")],
    )
    out = np.asarray(out).reshape(B, H, W, DIM)
    return out.astype(np.float32)


# revision 3
# speedup vs baseline: 1.1305x; 1.0738x over previous
"""Windowed cross-attention with contextual RPE, data-parallel over batch
across 8 NeuronCores.  v3: v2 + bf16-staged inputs, bf16-before-transpose,
and a no-max-subtraction softmax (logits here are O(1), exp cannot
overflow; correctness is checked against the fp32 reference).
"""

import numpy as np

import jax
import jax.numpy as jnp

WS = 7
NH = 12
DIM = 384
HD = DIM // NH
L = WS * WS
SCALE = HD ** (-0.5)
N_CORES = 8
NW = 8


def _relative_position_index() -> np.ndarray:
    coords = np.stack(np.meshgrid(np.arange(WS), np.arange(WS), indexing="ij"))
    flat = coords.reshape(2, -1)
    rel = flat[:, :, None] - flat[:, None, :]
    rel = rel.transpose(1, 2, 0).copy()
    rel[:, :, 0] += WS - 1
    rel[:, :, 1] += WS - 1
    rel[:, :, 0] *= 2 * WS - 1
    return rel.sum(-1)


_RPI = _relative_position_index()

_BF = jnp.bfloat16
_F32 = jnp.float32


def _window(t, b):
    t = t.reshape(b, NW, WS, NW, WS, DIM)
    t = t.transpose(0, 1, 3, 2, 4, 5)
    return t.reshape(b * NW * NW, L, DIM)


def _unwindow(t, b):
    t = t.reshape(b, NW, NW, WS, WS, DIM)
    t = t.transpose(0, 1, 3, 2, 4, 5)
    return t.reshape(b, NW * WS, NW * WS, DIM)


def _bmm(a, b):
    return jax.lax.dot_general(
        a, b,
        (((a.ndim - 1,), (1,)), (tuple(range(a.ndim - 2)), (0,))),
        preferred_element_type=_F32,
    )


def _core_fn(x, context, q_w, q_b, k_w, k_b, v_w, v_b, proj_w, proj_b,
             k_rpe_b, q_rpe_b, v_rpe_b):
    b = x.shape[0]
    bw = b * NW * NW

    xw = _window(x, b).reshape(bw * L, DIM)          # bf16
    cw = _window(context, b).reshape(bw * L, DIM)

    mm = lambda a, w: jax.lax.dot_general(
        a, w, (((1,), (0,)), ((), ())), preferred_element_type=_F32)
    q = (mm(xw, q_w) * SCALE + q_b * SCALE).astype(_BF)   # [bw*L, DIM]
    k = (mm(cw, k_w) + k_b).astype(_BF)
    v = (mm(cw, v_w) + v_b).astype(_BF)

    heads = lambda t: t.reshape(bw, L, NH, HD).transpose(0, 2, 1, 3)
    qh = heads(q)                                    # [bw, NH, L, HD] bf16
    kh = heads(k)
    vh = heads(v)

    qk = _bmm(qh.reshape(bw * NH, L, HD),
              kh.reshape(bw * NH, L, HD).transpose(0, 2, 1))

    q_hi = qh.transpose(1, 2, 0, 3).reshape(NH * L, bw, HD)
    qr = _bmm(q_hi, k_rpe_b)
    qr = qr.reshape(NH, L, bw, L).transpose(2, 0, 1, 3)

    k_hj = kh.transpose(1, 2, 0, 3).reshape(NH * L, bw, HD)
    kr = _bmm(k_hj, q_rpe_b)
    kr = kr.reshape(NH, L, bw, L).transpose(2, 0, 3, 1)

    logits = qk.reshape(bw, NH, L, L) + qr + kr
    e = jnp.exp(logits)                              # |logits| = O(1)
    attn = (e / e.sum(-1, keepdims=True)).astype(_BF)

    out1 = _bmm(attn.reshape(bw * NH, L, L), vh.reshape(bw * NH, L, HD))

    a_hi = attn.transpose(1, 2, 0, 3).reshape(NH * L, bw, L)
    out2 = _bmm(a_hi, v_rpe_b)
    out2 = out2.reshape(NH, L, bw, HD).transpose(2, 0, 1, 3)

    out = (out1.reshape(bw, NH, L, HD) + out2).astype(_BF)
    out = out.transpose(0, 2, 1, 3).reshape(bw * L, DIM)

    res = mm(out, proj_w) + proj_b
    return _unwindow(res.reshape(bw, L, DIM), b)


_PMAP = None


def _get_pmap():
    global _PMAP
    if _PMAP is None:
        _PMAP = jax.pmap(_core_fn, devices=jax.devices()[:N_CORES])
    return _PMAP


def _tile8(a):
    a = np.asarray(a)
    return np.broadcast_to(a, (N_CORES,) + a.shape)


def _prep_consts(rpe_table, q_w, q_b, kv_w, kv_b, proj_w, proj_b):
    import ml_dtypes

    rpe = np.asarray(rpe_table)[_RPI.reshape(-1)].reshape(L, L, NH, 3 * HD)
    q_rpe, k_rpe, v_rpe = np.split(rpe, 3, axis=-1)
    q_rpe = q_rpe * SCALE

    def as_bf16(a):
        return np.ascontiguousarray(a, np.float32).astype(ml_dtypes.bfloat16)

    k_rpe_b = as_bf16(k_rpe.transpose(2, 0, 3, 1).reshape(NH * L, HD, L))
    q_rpe_b = as_bf16(q_rpe.transpose(2, 1, 3, 0).reshape(NH * L, HD, L))
    v_rpe_b = as_bf16(v_rpe.transpose(2, 0, 1, 3).reshape(NH * L, L, HD))

    kv_w = np.asarray(kv_w)
    kv_b = np.asarray(kv_b)
    return dict(
        q_w=as_bf16(q_w), q_b=np.asarray(q_b, np.float32),
        k_w=as_bf16(kv_w[:, :DIM]), k_b=kv_b[:DIM].astype(np.float32),
        v_w=as_bf16(kv_w[:, DIM:]), v_b=kv_b[DIM:].astype(np.float32),
        proj_w=as_bf16(proj_w), proj_b=np.asarray(proj_b, np.float32),
        k_rpe_b=k_rpe_b, q_rpe_b=q_rpe_b, v_rpe_b=v_rpe_b,
    )


def _stage_inputs(x, context):
    import ml_dtypes

    B, H, W, _ = np.asarray(x).shape
    per = B // N_CORES
    xs = np.asarray(x).reshape(N_CORES, per, H, W, DIM).astype(ml_dtypes.bfloat16)
    cs = np.asarray(context).reshape(N_CORES, per, H, W, DIM).astype(
        ml_dtypes.bfloat16)
    return xs, cs


def kernel(x, context, rpe_table, q_w, q_b, kv_w, kv_b, proj_w, proj_b):
    x = np.asarray(x)
    B, H, W, _ = x.shape

    consts = _prep_consts(rpe_table, q_w, q_b, kv_w, kv_b, proj_w, proj_b)
    xs, cs = _stage_inputs(x, context)

    out = _get_pmap()(
        xs, cs,
        *[_tile8(consts[n]) for n in
          ("q_w", "q_b", "k_w", "k_b", "v_w", "v_b", "proj_w", "proj_b",
           "k_rpe_b", "q_rpe_b", "v_rpe_b")],
    )
    out = np.asarray(out).reshape(B, H, W, DIM)
    return out.astype(np.float32)
